# revision 47
# baseline (speedup 1.0000x reference)
"""Trainium2 Bass kernel for GPT-Neo style causal attention.

reference:
    scores = q @ k.T              (no 1/sqrt(d) scaling), fp32
    scores = where(causal, scores, -inf)
    attn   = softmax(scores, -1)
    attn   = attn * ctx_mask[b, None, None, :]
    out    = attn @ v

Shapes: B=2, H=16, S=2048, D=128 fp32. Sharded over 8 cores by (b*h) —
4 heads per core; each core's heads belong to one batch, so one
ctx_mask row per core.

Per-core algorithm (T-layout softmax, no transposes of the attn matrix):
  - Q,K,V are loaded via GPSIMD software-DGE cast-DMAs: the DMA path
    converts fp32 HBM -> fp16 (Q,K) / bf16 (V, strided into the 129-wide
    V' tile) while moving the data, so no compute engine touches the
    input casts and the loads live on the Pool DMA queue where the sync
    queue's transposes/stores can never delay them.  The Pool queue is
    FIFO, which self-orders each head's K before Q before V and head
    bh's loads before head bh+1's.
  - one xbar DMA transpose per tensor (sync queue, slots t==8/10 of the
    previous head's strip loop) -> interleaved [Q^T | K^T] tile [d, s]
    fp16.  NOTE an xbar transpose waits out ALL in-flight DMAs of its
    queue, so only the (idle-ish) sync queue may carry them, and stores
    are emitted before them.
  - per key-block t: scoresT[keys,q] = KT_blk.T @ QT  (only q >= t*128,
    512-col segments aligned to PSUM banks)
  - one exp() per strip on ScalarE with per-partition bias ln(ctx_mask):
    expT = exp(s + ln(cm_key)) = exp(s)*cm_key  -> bf16 (the ctx-mask
    multiply costs nothing).  Causal diag via additive -3e38 mask on the
    diagonal block in PSUM pre-exp (PE-side: trineg^T @ I accumulate).
  - AV: out_psum[q, 0:129] = sum_kb expT_blk.T @ [V | 1/cm] (bf16,
    fp32 PSUM accum).  Column 128 accumulates exp*cm*(1/cm) = exp,
    i.e. the pre-ctx-mask softmax denominator -> reciprocal + scale.
    Two AV regions share each [128,512] PSUM bank tile so a new
    av_block's WAR (on the DVE reciprocal/scale of the old occupant)
    reaches ~4 blocks back instead of 2 and the PE never stalls on the
    DVE queue.
  - cm clamped at 1e-30 so cm=0 stays exact.

No max-subtraction is needed: |scores| <~ 95 so exp() stays inside fp32/
bf16 range after the -16 bias shift (which cancels in the softmax ratio).
A dummy bf16 matmul burst at the start (hidden under the first load +
transpose chain) warms the PE p-state/HAM clock.

Pipelining: prep(bh+1) swdge loads are emitted at the top of
compute(bh); its transposes at slots t==8/10; the previous head's output
stores at t==4/6 (before the transposes, so the xbar barrier cannot
couple them).  The next head's first NPRE strips are emitted before this
head's last AV so ScalarE never drains at head boundaries.  The last
head's stores are chunked right behind the AV drain so the final store
is tiny.
"""

from contextlib import ExitStack

import numpy as np

import concourse.bass as bass
import concourse.mybir as mybir
import concourse.tile as tile
from concourse.bass_utils import run_bass_kernel_spmd
from concourse.masks import make_identity, make_upper_triangular

F32 = mybir.dt.float32
F16 = mybir.dt.float16
BF16 = mybir.dt.bfloat16

B, H, S, D = 2, 16, 2048, 128
NCORES = 8
NBH = (B * H) // NCORES  # heads per core


def _legalize_waits(nc):
    """This container's walrus accepts at most 1 sync wait per instruction
    (2 for EventSemaphore). Hoist extra waits onto same-engine NoOps
    inserted immediately before the offending instruction (semantically
    identical: all waits still complete before it executes)."""
    n = 0
    ctr = [0]
    for f in nc.m.functions:
        for bb in f.blocks:
            out = []
            dirty = False
            for inst in bb.instructions:
                si = inst.sync_info
                cap = 2 if isinstance(inst, mybir.InstEventSemaphore) else 1
                if si is not None and len(si.on_wait) > cap:
                    waits = list(si.on_wait)
                    extra, keep = waits[:-cap], waits[-cap:]
                    for w in extra:
                        ctr[0] += 1
                        nop = mybir.InstNoOp(
                            name=f"waitsplit-{ctr[0]}",
                            ins=[],
                            outs=[],
                            engine=inst.engine,
                            sync_info=mybir.SyncInfo(on_wait=[w], on_update=[]),
                        )
                        nc.register_instruction(nop, overwrite=True)
                        out.append(nop)
                    inst.sync_info = mybir.SyncInfo(
                        on_wait=keep, on_update=list(si.on_update)
                    )
                    dirty = True
                    n += 1
                out.append(inst)
            if dirty:
                bb.instructions = out
    return n


def build_nc(nbh=NBH, s=S, d=D, num_devices=NCORES, nwarm=150):
    SB = s // 128  # 128-row blocks along the sequence
    nc = bass.Bass("TRN2", target_bir_lowering=False, debug=False,
                   num_devices=num_devices)
    q = nc.dram_tensor("q", [nbh, s, d], F32, kind="ExternalInput")
    k = nc.dram_tensor("k", [nbh, s, d], F32, kind="ExternalInput")
    v = nc.dram_tensor("v", [nbh, s, d], F32, kind="ExternalInput")
    cm = nc.dram_tensor("cm", [s], F32, kind="ExternalInput")
    o = nc.dram_tensor("out", [nbh, s, d], F32, kind="ExternalOutput")

    EXPFN = mybir.ActivationFunctionType.Exp
    LNFN = mybir.ActivationFunctionType.Ln

    with tile.TileContext(nc) as tc, ExitStack() as ctx:
        consts = ctx.enter_context(tc.tile_pool(name="consts", bufs=1))
        stage = ctx.enter_context(tc.tile_pool(name="stage", bufs=1))
        # bufs=1: head bh+1's swdge K/Q cast-loads reuse head bh's
        # buffers, so their writes carry a REAL WAR dependency on
        # T(K,bh)/T(Q,bh) — the slow swdge transfers cannot start until
        # the transposes have consumed the previous tiles.  (An
        # artificial Pool-queue "gate" read does NOT work: the tile
        # scheduler reorders ready instructions past it.)  Steady-state
        # cost is nil: T(K,bh) completes during compute(bh-1).
        hpool = ctx.enter_context(tc.tile_pool(name="hpool", bufs=1))
        tpool = ctx.enter_context(tc.tile_pool(name="tpool", bufs=2))
        vpool = ctx.enter_context(tc.tile_pool(name="vpool", bufs=2))
        epool = ctx.enter_context(tc.tile_pool(name="epool", bufs=1))
        # expT[0..NPRE-1] double-buffered: the next head's first NPRE
        # strips (its LONGEST exp work) are computed during this head's
        # AV tail so ScalarE never drains at head boundaries.
        NPRE = 4
        epool2 = ctx.enter_context(tc.tile_pool(name="epool2", bufs=2))
        opool = ctx.enter_context(tc.tile_pool(name="opool", bufs=3))
        small = ctx.enter_context(tc.tile_pool(name="small", bufs=4))
        psum = ctx.enter_context(tc.tile_pool(name="psum", bufs=2, space="PSUM"))
        psav = ctx.enter_context(tc.tile_pool(name="psav", bufs=2, space="PSUM"))

        qap, kap, vap, oap = q.ap(), k.ap(), v.ap(), o.ap()

        pending_vcast = [None]

        def loads(bh):
            """fp32 HBM -> fp16/bf16 SBUF cast loads on the Pool
            (software DGE) queue.  K first: T(K) gates the next head's
            first QK strip's weight.  Head 1's V goes through the
            Activation hwdge queue + a DVE cast (emitted later, slot
            t==4) — its swdge transfer would otherwise start unngated at
            t~8us and trample head 0's fast loads; V2+ are WAR-gated by
            vpool bufs=2 (their buffer is still being read by head
            bh-1's AV drain), which is a real dependency."""
            kh = hpool.tile([128, SB, d], F16, tag="kh")
            qh = hpool.tile([128, SB, d], F16, tag="qh")
            vp = vpool.tile([128, SB, d + 1], BF16, tag="vp", name=f"vp_{bh}")
            nc.gpsimd.dma_start(out=kh, in_=kap[bh].rearrange("(sb p) d -> p sb d", p=128))
            nc.gpsimd.dma_start(out=qh, in_=qap[bh].rearrange("(sb p) d -> p sb d", p=128))
            if bh == 1:
                vn = stage.tile([128, SB, d], F32, tag="vn", name="vn_1")
                nc.scalar.dma_start(out=vn, in_=vap[bh].rearrange("(sb p) d -> p sb d", p=128))
                pending_vcast[0] = (vp, vn)
            else:
                nc.gpsimd.dma_start(out=vp[:, :, 0:d], in_=vap[bh].rearrange("(sb p) d -> p sb d", p=128))
            return kh, qh, vp

        # Head 0 goes through the fast hwdge fp32 path + DVE casts on
        # the sync queue: the swdge cast-DMA path is several times
        # slower per tensor, which steady-state pipelining hides but the
        # cold start pays in full.
        cmt = consts.tile([128, SB], F32)
        nc.sync.dma_start(out=cmt, in_=cm.ap().rearrange("(sb p) -> p sb", p=128))
        kn0 = stage.tile([128, SB, d], F32, tag="kn")
        qn0 = stage.tile([128, SB, d], F32, tag="qn")
        vn0 = stage.tile([128, SB, d], F32, tag="vn")
        # Half-granular loads, K stream on the sync hwdge queue and the
        # Q stream + V0 on the Activation hwdge queue, so the two
        # streams transfer in PARALLEL and the cast->transpose chain
        # pipelines: the PE can start strip 0's first segments as soon
        # as the first Q-half transpose lands (region-level deps).
        HB = SB // 2
        for h0, h1 in ((0, HB), (HB, SB)):
            nc.sync.dma_start(out=kn0[:, h0:h1, :],
                              in_=kap[0][h0 * 128:h1 * 128].rearrange("(sb p) d -> p sb d", p=128))
            nc.scalar.dma_start(out=qn0[:, h0:h1, :],
                              in_=qap[0][h0 * 128:h1 * 128].rearrange("(sb p) d -> p sb d", p=128))
        # V0 after Q on the Activation queue: it is only needed at the
        # first av_block (~2us after the first exp).
        nc.scalar.dma_start(out=vn0, in_=vap[0].rearrange("(sb p) d -> p sb d", p=128))

        ident = consts.tile([128, 128], F32)
        make_identity(nc, ident)
        identb = consts.tile([128, 128], BF16)
        nc.vector.tensor_copy(identb, ident)
        # additive causal mask for the diagonal block, accumulated into the
        # scores PSUM by the PE itself: matmul(trinegT, I) adds
        # trinegT.T[k, q] = -3e38 for q < k.
        trinegT = consts.tile([128, 128], F32)
        make_upper_triangular(nc, trinegT, val=-3e38, diag=False)
        trinegTb = consts.tile([128, 128], BF16)
        nc.vector.tensor_copy(trinegTb, trinegT)

        # ctx-mask pipeline: cmc = max(cm, 1e-30); lncm = ln(cmc) - 16
        # (exp bias); invcb = 1/cmc in bf16 (denominator column of V')
        cmc = consts.tile([128, SB], F32)
        nc.vector.tensor_scalar_max(cmc, cmt, 1e-30)
        lncm = consts.tile([128, SB], F32)
        nc.scalar.activation(lncm, cmc, LNFN)
        nc.vector.tensor_scalar_add(lncm, lncm, -16.0)
        invc = consts.tile([128, SB], F32)
        nc.vector.reciprocal(invc, cmc)
        invcb = consts.tile([128, SB], BF16)
        nc.vector.tensor_copy(invcb, invc)

        # head 0's DVE casts, half-granular to match the loads (after
        # the tiny cm-pipeline/mask DVE ops so those clear the queue
        # while the loads are still in flight)
        kh0 = hpool.tile([128, SB, d], F16, tag="kh")
        qh0 = hpool.tile([128, SB, d], F16, tag="qh")
        vp0 = vpool.tile([128, SB, d + 1], BF16, tag="vp")
        for h0, h1 in ((0, HB), (HB, SB)):
            nc.vector.tensor_copy(kh0[:, h0:h1, :], kn0[:, h0:h1, :])
            nc.vector.tensor_copy(qh0[:, h0:h1, :], qn0[:, h0:h1, :])
        nc.vector.tensor_copy(vp0[:, :, 0:d], vn0)
        nxt_ld = (kh0, qh0, vp0)

        # Dummy bf16 matmuls (values irrelevant) to warm the PE clock
        # while the first loads + transposes are in flight; memset-only
        # dep so the burst starts at t~0.
        wpw = consts.tile([128, 128], BF16)
        nc.vector.memset(wpw, 1.0)
        wps = psav.tile([128, 256], F32, tag="av")
        for _ in range(nwarm):
            nc.tensor.matmul(wps[:, 0:128], wpw, wpw, start=True, stop=True)

        def transposes(ld):
            """xbar transposes (sync hwdge queue): interleaved
            [Q^T | K^T] [d, s] fp16. qkt[:, sb, 0, :] = Q^T,
            qkt[:, sb, 1, :] = K^T."""
            kh, qh, vp = ld
            qkt = tpool.tile([128, SB, 2, 128], F16, tag="qkt")
            nc.sync.dma_start_transpose(out=qkt[:, :, 1, :], in_=kh)
            nc.sync.dma_start_transpose(out=qkt[:, :, 0, :], in_=qh)
            return qkt, vp

        def store_chunk(sbh, sostage, g0, g1):
            nc.sync.dma_start(
                out=oap[sbh][g0 * 128:g1 * 128].rearrange(
                    "(sb p) d -> p sb d", p=128),
                in_=sostage[:, g0:g1, :],
            )

        def make_expT(bh):
            return [
                (epool2 if kb < NPRE else epool).tile(
                    [128, s], BF16, tag=f"expT{kb}", name=f"expT{kb}_{bh}")
                for kb in range(SB)
            ]

        def do_strip(t, qkt_, expT_):
            for (lo, hi) in (((t * 128) // 512 * 512, min(((t * 128) // 512 * 512) + 1536, s)),
                             (min(((t * 128) // 512 * 512) + 1536, s), s)):
                if lo >= hi:
                    continue
                sc = psum.tile([128, 1536], F32, tag="ps")
                q0 = max(t * 128, lo)
                qstart = q0
                while qstart < hi:
                    seg = min(512 - (qstart % 512), hi - qstart)
                    b0, b1 = qstart // 128, (qstart + seg) // 128
                    diag = qstart == t * 128
                    nc.tensor.matmul(
                        sc[:, qstart - lo:qstart - lo + seg],
                        qkt_[:, t, 1, :],
                        qkt_[:, b0:b1, 0, :],
                        start=True,
                        stop=not diag,
                    )
                    if diag:
                        nc.tensor.matmul(
                            sc[:, qstart - lo:qstart - lo + 128],
                            trinegTb,
                            identb,
                            start=False,
                            stop=True,
                            skip_group_check=True,
                        )
                    qstart += seg
                # exp(s - 16 + ln(cm_key)) -> bf16
                nc.scalar.activation(expT_[t][:, q0:hi], sc[:, q0 - lo:hi - lo],
                                     EXPFN, bias=lncm[:, t:t + 1])

        # first head's transposes: half-granular, on the sync queue,
        # whose only in-flight DMAs are the (early-finishing) K halves
        # — so the xbar barrier costs nothing and each half fires the
        # moment its cast lands.  Q-half-0 second: it gates the PE.
        qkt0 = tpool.tile([128, SB, 2, 128], F16, tag="qkt")
        for h0, h1 in ((0, HB), (HB, SB)):
            nc.sync.dma_start_transpose(out=qkt0[:, h0:h1, 1, :], in_=kh0[:, h0:h1, :])
            nc.sync.dma_start_transpose(out=qkt0[:, h0:h1, 0, :], in_=qh0[:, h0:h1, :])
        nxt = (qkt0, vp0)

        prev = None
        expT = make_expT(0)
        pre_done = 0
        for bh in range(nbh):
            qkt, vp = nxt
            # V' denominator column for THIS head, written here: a
            # slot-emitted write in the previous head's loop would wait
            # on the V swdge transfer from inside the DVE queue and
            # head-of-line-block every scale op behind it.
            nc.vector.tensor_copy(vp[:, :, d], invcb)
            nxt_ld = loads(bh + 1) if bh + 1 < nbh else None

            ostage = opool.tile([128, SB, d], F32, tag="ostage")

            def av_block(qb, expT_=expT):
                # expT_ bound at def time: the tail av_block(SB-1) runs
                # after `expT` has been swapped to the next head's list.
                av = psav.tile([128, 256], F32, tag="av")
                for kb in range(qb + 1):
                    nc.tensor.matmul(
                        av[:, 0:d + 1],
                        expT_[kb][:, qb * 128:(qb + 1) * 128],
                        vp[:, kb, :],
                        start=(kb == 0),
                        stop=(kb == qb),
                    )
                rec = small.tile([128, 1], F32, tag="rec")
                nc.vector.reciprocal(rec, av[:, d:d + 1])
                nc.vector.tensor_scalar_mul(ostage[:, qb, :], av[:, 0:d], rec)

            last = bh == nbh - 1
            # AVs for strips this head inherited from the previous head's
            # tail: their exps are in flight or done, and these cheap early
            # AV blocks fill the PE while ScalarE chews the long strips.
            for qb in range(max(pre_done - 1, 0)):
                av_block(qb)
            for t in range(pre_done, SB):
                do_strip(t, qkt, expT)
                if t >= 1:
                    av_block(t - 1)  # one step behind so PE never waits on exp
                # NOTE: heads after the first start this loop at
                # t=pre_done(=4), so all slots here must be >= 4.
                # Stores before the transposes: the xbar barrier waits
                # out every in-flight DMA of the sync queue.
                if t == 4 and pending_vcast[0] is not None:
                    vpp, vnn = pending_vcast[0]
                    pending_vcast[0] = None
                    nc.vector.tensor_copy(vpp[:, :, 0:d], vnn)
                if prev is not None:
                    if t == 4:
                        store_chunk(prev[0], prev[1], 0, SB // 2)
                    elif t == 6:
                        store_chunk(prev[0], prev[1], SB // 2, SB)
                if nxt_ld is not None:
                    if t == 8:
                        nxt = transposes(nxt_ld)
                # last head: chunked stores right behind the AV drain so
                # the final store is tiny.  At step t, av_block(t-1) has
                # been emitted, so blocks [0, t) of ostage are in flight.
                if last:
                    if t == 9:
                        store_chunk(bh, ostage, 0, SB // 2)
                    elif t == 15:
                        store_chunk(bh, ostage, SB // 2, SB - 1)
            # tail: the next head's first NPRE strips go out BEFORE the
            # last AV so their (long) exps overlap this head's AV drain;
            # they write the OTHER epool2 buffers, so no clash with this
            # head's remaining AV reads of expT[0..NPRE-1].
            if not last:
                expT_next = make_expT(bh + 1)
                for t in range(NPRE):
                    do_strip(t, nxt[0], expT_next)
                expT, pre_done = expT_next, NPRE
            av_block(SB - 1)
            if last:
                store_chunk(bh, ostage, SB - 1, SB)
            prev = (bh, ostage)

    _legalize_waits(nc)
    return nc


_nc_cache = {}


def _get_nc():
    key = (NBH, S, D)
    if key not in _nc_cache:
        _nc_cache[key] = build_nc()
    return _nc_cache[key]


def kernel(query, key, value, ctx_mask):
    q = np.ascontiguousarray(query, dtype=np.float32).reshape(B * H, S, D)
    k = np.ascontiguousarray(key, dtype=np.float32).reshape(B * H, S, D)
    v = np.ascontiguousarray(value, dtype=np.float32).reshape(B * H, S, D)
    cmf = np.ascontiguousarray(ctx_mask, dtype=np.float32)

    in_maps = []
    for c in range(NCORES):
        lo = c * NBH
        in_maps.append({
            "q": q[lo:lo + NBH],
            "k": k[lo:lo + NBH],
            "v": v[lo:lo + NBH],
            "cm": cmf[(lo // H)],
        })
    nc = _get_nc()
    res = run_bass_kernel_spmd(nc, in_maps, list(range(NCORES)))
    outs = [res.results[c]["out"] for c in range(NCORES)]
    return np.concatenate(outs, axis=0).reshape(B, H, S, D).astype(np.float32)


# revision 48
# speedup vs baseline: 1.1613x; 1.1613x over previous
"""Trainium2 Bass kernel for GPT-Neo style causal attention.

reference:
    scores = q @ k.T              (no 1/sqrt(d) scaling), fp32
    scores = where(causal, scores, -inf)
    attn   = softmax(scores, -1)
    attn   = attn * ctx_mask[b, None, None, :]
    out    = attn @ v

Shapes: B=2, H=16, S=2048, D=128 fp32. Sharded over 8 cores by (b*h) —
4 heads per core; each core's heads belong to one batch, so one
ctx_mask row per core.

Per-core algorithm (T-layout softmax, no transposes of the attn matrix):
  - Q,K,V are loaded via GPSIMD software-DGE cast-DMAs: the DMA path
    converts fp32 HBM -> fp16 (Q,K) / bf16 (V, strided into the 129-wide
    V' tile) while moving the data, so no compute engine touches the
    input casts and the loads live on the Pool DMA queue where the sync
    queue's transposes/stores can never delay them.  The Pool queue is
    FIFO, which self-orders each head's K before Q before V and head
    bh's loads before head bh+1's.
  - one xbar DMA transpose per tensor (sync queue, slots t==8/10 of the
    previous head's strip loop) -> interleaved [Q^T | K^T] tile [d, s]
    fp16.  NOTE an xbar transpose waits out ALL in-flight DMAs of its
    queue, so only the (idle-ish) sync queue may carry them, and stores
    are emitted before them.
  - per key-block t: scoresT[keys,q] = KT_blk.T @ QT  (only q >= t*128,
    512-col segments aligned to PSUM banks)
  - one exp() per strip on ScalarE with per-partition bias ln(ctx_mask):
    expT = exp(s + ln(cm_key)) = exp(s)*cm_key  -> bf16 (the ctx-mask
    multiply costs nothing).  Causal diag via additive -3e38 mask on the
    diagonal block in PSUM pre-exp (PE-side: trineg^T @ I accumulate).
  - AV: out_psum[q, 0:129] = sum_kb expT_blk.T @ [V | 1/cm] (bf16,
    fp32 PSUM accum).  Column 128 accumulates exp*cm*(1/cm) = exp,
    i.e. the pre-ctx-mask softmax denominator -> reciprocal + scale.
    Two AV regions share each [128,512] PSUM bank tile so a new
    av_block's WAR (on the DVE reciprocal/scale of the old occupant)
    reaches ~4 blocks back instead of 2 and the PE never stalls on the
    DVE queue.
  - cm clamped at 1e-30 so cm=0 stays exact.

No max-subtraction is needed: |scores| <~ 95 so exp() stays inside fp32/
bf16 range after the -16 bias shift (which cancels in the softmax ratio).
A dummy bf16 matmul burst at the start (hidden under the first load +
transpose chain) warms the PE p-state/HAM clock.

Pipelining: prep(bh+1) swdge loads are emitted at the top of
compute(bh); its transposes at slots t==8/10; the previous head's output
stores at t==4/6 (before the transposes, so the xbar barrier cannot
couple them).  The next head's first NPRE strips are emitted before this
head's last AV so ScalarE never drains at head boundaries.  The last
head's stores are chunked right behind the AV drain so the final store
is tiny.
"""

from contextlib import ExitStack

import numpy as np

import concourse.bass as bass
import concourse.mybir as mybir
import concourse.tile as tile
from concourse.bass_utils import run_bass_kernel_spmd
from concourse.masks import make_identity, make_upper_triangular

F32 = mybir.dt.float32
F16 = mybir.dt.float16
BF16 = mybir.dt.bfloat16

B, H, S, D = 2, 16, 2048, 128
NCORES = 8
NBH = (B * H) // NCORES  # heads per core


def _legalize_waits(nc):
    """This container's walrus accepts at most 1 sync wait per instruction
    (2 for EventSemaphore). Hoist extra waits onto same-engine NoOps
    inserted immediately before the offending instruction (semantically
    identical: all waits still complete before it executes)."""
    n = 0
    ctr = [0]
    for f in nc.m.functions:
        for bb in f.blocks:
            out = []
            dirty = False
            for inst in bb.instructions:
                si = inst.sync_info
                cap = 2 if isinstance(inst, mybir.InstEventSemaphore) else 1
                if si is not None and len(si.on_wait) > cap:
                    waits = list(si.on_wait)
                    extra, keep = waits[:-cap], waits[-cap:]
                    for w in extra:
                        ctr[0] += 1
                        nop = mybir.InstNoOp(
                            name=f"waitsplit-{ctr[0]}",
                            ins=[],
                            outs=[],
                            engine=inst.engine,
                            sync_info=mybir.SyncInfo(on_wait=[w], on_update=[]),
                        )
                        nc.register_instruction(nop, overwrite=True)
                        out.append(nop)
                    inst.sync_info = mybir.SyncInfo(
                        on_wait=keep, on_update=list(si.on_update)
                    )
                    dirty = True
                    n += 1
                out.append(inst)
            if dirty:
                bb.instructions = out
    return n


def build_nc(nbh=NBH, s=S, d=D, num_devices=NCORES, nwarm=115):
    # nwarm sizing: the warmup burst shares the PE queue with the first
    # real strip, so an oversized burst DELAYS it.  With the parallel
    # dual-queue startup the cold first-QK lands at ~13us while 150
    # warmup matmuls run to ~15us; 115 ends ~12.6us cold (seamless
    # p-state handoff) and merely leaves a short already-DMA-bound hole
    # on hot runs.
    SB = s // 128  # 128-row blocks along the sequence
    nc = bass.Bass("TRN2", target_bir_lowering=False, debug=False,
                   num_devices=num_devices)
    q = nc.dram_tensor("q", [nbh, s, d], F32, kind="ExternalInput")
    k = nc.dram_tensor("k", [nbh, s, d], F32, kind="ExternalInput")
    v = nc.dram_tensor("v", [nbh, s, d], F32, kind="ExternalInput")
    cm = nc.dram_tensor("cm", [s], F32, kind="ExternalInput")
    o = nc.dram_tensor("out", [nbh, s, d], F32, kind="ExternalOutput")

    EXPFN = mybir.ActivationFunctionType.Exp
    LNFN = mybir.ActivationFunctionType.Ln

    with tile.TileContext(nc) as tc, ExitStack() as ctx:
        consts = ctx.enter_context(tc.tile_pool(name="consts", bufs=1))
        stage = ctx.enter_context(tc.tile_pool(name="stage", bufs=1))
        # bufs=1: head bh+1's swdge K/Q cast-loads reuse head bh's
        # buffers, so their writes carry a REAL WAR dependency on
        # T(K,bh)/T(Q,bh) — the slow swdge transfers cannot start until
        # the transposes have consumed the previous tiles.  (An
        # artificial Pool-queue "gate" read does NOT work: the tile
        # scheduler reorders ready instructions past it.)  Steady-state
        # cost is nil: T(K,bh) completes during compute(bh-1).
        hpool = ctx.enter_context(tc.tile_pool(name="hpool", bufs=1))
        tpool = ctx.enter_context(tc.tile_pool(name="tpool", bufs=2))
        vpool = ctx.enter_context(tc.tile_pool(name="vpool", bufs=2))
        epool = ctx.enter_context(tc.tile_pool(name="epool", bufs=1))
        # expT[0..NPRE-1] double-buffered: the next head's first NPRE
        # strips (its LONGEST exp work) are computed during this head's
        # AV tail so ScalarE never drains at head boundaries.
        NPRE = 4
        epool2 = ctx.enter_context(tc.tile_pool(name="epool2", bufs=2))
        opool = ctx.enter_context(tc.tile_pool(name="opool", bufs=3))
        small = ctx.enter_context(tc.tile_pool(name="small", bufs=4))
        psum = ctx.enter_context(tc.tile_pool(name="psum", bufs=2, space="PSUM"))
        psav = ctx.enter_context(tc.tile_pool(name="psav", bufs=2, space="PSUM"))

        qap, kap, vap, oap = q.ap(), k.ap(), v.ap(), o.ap()

        pending_vcast = [None]

        def loads(bh):
            """fp32 HBM -> fp16/bf16 SBUF cast loads on the Pool
            (software DGE) queue.  K first: T(K) gates the next head's
            first QK strip's weight.  Head 1's V goes through the
            Activation hwdge queue + a DVE cast (emitted later, slot
            t==4) — its swdge transfer would otherwise start unngated at
            t~8us and trample head 0's fast loads; V2+ are WAR-gated by
            vpool bufs=2 (their buffer is still being read by head
            bh-1's AV drain), which is a real dependency."""
            kh = hpool.tile([128, SB, d], F16, tag="kh")
            qh = hpool.tile([128, SB, d], F16, tag="qh")
            vp = vpool.tile([128, SB, d + 1], BF16, tag="vp", name=f"vp_{bh}")
            nc.gpsimd.dma_start(out=kh, in_=kap[bh].rearrange("(sb p) d -> p sb d", p=128))
            nc.gpsimd.dma_start(out=qh, in_=qap[bh].rearrange("(sb p) d -> p sb d", p=128))
            if bh == 1:
                vn = stage.tile([128, SB, d], F32, tag="vn", name="vn_1")
                nc.scalar.dma_start(out=vn, in_=vap[bh].rearrange("(sb p) d -> p sb d", p=128))
                pending_vcast[0] = (vp, vn)
            else:
                nc.gpsimd.dma_start(out=vp[:, :, 0:d], in_=vap[bh].rearrange("(sb p) d -> p sb d", p=128))
            return kh, qh, vp

        # Head 0 goes through the fast hwdge fp32 path + DVE casts on
        # the sync queue: the swdge cast-DMA path is several times
        # slower per tensor, which steady-state pipelining hides but the
        # cold start pays in full.
        cmt = consts.tile([128, SB], F32)
        nc.sync.dma_start(out=cmt, in_=cm.ap().rearrange("(sb p) -> p sb", p=128))
        kn0 = stage.tile([128, SB, d], F32, tag="kn")
        qn0 = stage.tile([128, SB, d], F32, tag="qn")
        vn0 = stage.tile([128, SB, d], F32, tag="vn")
        # Half-granular loads, K stream on the sync hwdge queue and the
        # Q stream + V0 on the Activation hwdge queue, so the two
        # streams transfer in PARALLEL and the cast->transpose chain
        # pipelines: the PE can start strip 0's first segments as soon
        # as the first Q-half transpose lands (region-level deps).
        HB = SB // 2
        for h0, h1 in ((0, HB), (HB, SB)):
            nc.sync.dma_start(out=kn0[:, h0:h1, :],
                              in_=kap[0][h0 * 128:h1 * 128].rearrange("(sb p) d -> p sb d", p=128))
            nc.scalar.dma_start(out=qn0[:, h0:h1, :],
                              in_=qap[0][h0 * 128:h1 * 128].rearrange("(sb p) d -> p sb d", p=128))
        # V0 after Q on the Activation queue: it is only needed at the
        # first av_block (~2us after the first exp).
        nc.scalar.dma_start(out=vn0, in_=vap[0].rearrange("(sb p) d -> p sb d", p=128))

        ident = consts.tile([128, 128], F32)
        make_identity(nc, ident)
        identb = consts.tile([128, 128], BF16)
        nc.vector.tensor_copy(identb, ident)
        # additive causal mask for the diagonal block, accumulated into the
        # scores PSUM by the PE itself: matmul(trinegT, I) adds
        # trinegT.T[k, q] = -3e38 for q < k.
        trinegT = consts.tile([128, 128], F32)
        make_upper_triangular(nc, trinegT, val=-3e38, diag=False)
        trinegTb = consts.tile([128, 128], BF16)
        nc.vector.tensor_copy(trinegTb, trinegT)

        # ctx-mask pipeline: cmc = max(cm, 1e-30); lncm = ln(cmc) - 16
        # (exp bias); invcb = 1/cmc in bf16 (denominator column of V')
        cmc = consts.tile([128, SB], F32)
        nc.vector.tensor_scalar_max(cmc, cmt, 1e-30)
        lncm = consts.tile([128, SB], F32)
        nc.scalar.activation(lncm, cmc, LNFN)
        nc.vector.tensor_scalar_add(lncm, lncm, -16.0)
        invc = consts.tile([128, SB], F32)
        nc.vector.reciprocal(invc, cmc)
        invcb = consts.tile([128, SB], BF16)
        nc.vector.tensor_copy(invcb, invc)

        # head 0's DVE casts, half-granular to match the loads (after
        # the tiny cm-pipeline/mask DVE ops so those clear the queue
        # while the loads are still in flight)
        kh0 = hpool.tile([128, SB, d], F16, tag="kh")
        qh0 = hpool.tile([128, SB, d], F16, tag="qh")
        vp0 = vpool.tile([128, SB, d + 1], BF16, tag="vp")
        for h0, h1 in ((0, HB), (HB, SB)):
            nc.vector.tensor_copy(kh0[:, h0:h1, :], kn0[:, h0:h1, :])
            nc.vector.tensor_copy(qh0[:, h0:h1, :], qn0[:, h0:h1, :])
        nc.vector.tensor_copy(vp0[:, :, 0:d], vn0)
        nxt_ld = (kh0, qh0, vp0)

        # Dummy bf16 matmuls (values irrelevant) to warm the PE clock
        # while the first loads + transposes are in flight; memset-only
        # dep so the burst starts at t~0.
        wpw = consts.tile([128, 128], BF16)
        nc.vector.memset(wpw, 1.0)
        wps = psav.tile([128, 256], F32, tag="av")
        for _ in range(nwarm):
            nc.tensor.matmul(wps[:, 0:128], wpw, wpw, start=True, stop=True)

        def transposes(ld):
            """xbar transposes (sync hwdge queue): interleaved
            [Q^T | K^T] [d, s] fp16. qkt[:, sb, 0, :] = Q^T,
            qkt[:, sb, 1, :] = K^T."""
            kh, qh, vp = ld
            qkt = tpool.tile([128, SB, 2, 128], F16, tag="qkt")
            nc.sync.dma_start_transpose(out=qkt[:, :, 1, :], in_=kh)
            nc.sync.dma_start_transpose(out=qkt[:, :, 0, :], in_=qh)
            return qkt, vp

        def store_chunk(sbh, sostage, g0, g1):
            nc.sync.dma_start(
                out=oap[sbh][g0 * 128:g1 * 128].rearrange(
                    "(sb p) d -> p sb d", p=128),
                in_=sostage[:, g0:g1, :],
            )

        def make_expT(bh):
            return [
                (epool2 if kb < NPRE else epool).tile(
                    [128, s], BF16, tag=f"expT{kb}", name=f"expT{kb}_{bh}")
                for kb in range(SB)
            ]

        def do_strip(t, qkt_, expT_):
            for (lo, hi) in (((t * 128) // 512 * 512, min(((t * 128) // 512 * 512) + 1536, s)),
                             (min(((t * 128) // 512 * 512) + 1536, s), s)):
                if lo >= hi:
                    continue
                sc = psum.tile([128, 1536], F32, tag="ps")
                q0 = max(t * 128, lo)
                qstart = q0
                while qstart < hi:
                    seg = min(512 - (qstart % 512), hi - qstart)
                    b0, b1 = qstart // 128, (qstart + seg) // 128
                    diag = qstart == t * 128
                    nc.tensor.matmul(
                        sc[:, qstart - lo:qstart - lo + seg],
                        qkt_[:, t, 1, :],
                        qkt_[:, b0:b1, 0, :],
                        start=True,
                        stop=not diag,
                    )
                    if diag:
                        nc.tensor.matmul(
                            sc[:, qstart - lo:qstart - lo + 128],
                            trinegTb,
                            identb,
                            start=False,
                            stop=True,
                            skip_group_check=True,
                        )
                    qstart += seg
                # exp(s - 16 + ln(cm_key)) -> bf16
                nc.scalar.activation(expT_[t][:, q0:hi], sc[:, q0 - lo:hi - lo],
                                     EXPFN, bias=lncm[:, t:t + 1])

        # first head's transposes: half-granular, on the sync queue,
        # whose only in-flight DMAs are the (early-finishing) K halves
        # — so the xbar barrier costs nothing and each half fires the
        # moment its cast lands.  Q-half-0 second: it gates the PE.
        qkt0 = tpool.tile([128, SB, 2, 128], F16, tag="qkt")
        for h0, h1 in ((0, HB), (HB, SB)):
            nc.sync.dma_start_transpose(out=qkt0[:, h0:h1, 1, :], in_=kh0[:, h0:h1, :])
            nc.sync.dma_start_transpose(out=qkt0[:, h0:h1, 0, :], in_=qh0[:, h0:h1, :])
        nxt = (qkt0, vp0)

        prev = None
        expT = make_expT(0)
        pre_done = 0
        for bh in range(nbh):
            qkt, vp = nxt
            # V' denominator column for THIS head, written here: a
            # slot-emitted write in the previous head's loop would wait
            # on the V swdge transfer from inside the DVE queue and
            # head-of-line-block every scale op behind it.
            nc.vector.tensor_copy(vp[:, :, d], invcb)
            nxt_ld = loads(bh + 1) if bh + 1 < nbh else None

            ostage = opool.tile([128, SB, d], F32, tag="ostage")

            def av_block(qb, expT_=expT):
                # expT_ bound at def time: the tail av_block(SB-1) runs
                # after `expT` has been swapped to the next head's list.
                av = psav.tile([128, 256], F32, tag="av")
                for kb in range(qb + 1):
                    nc.tensor.matmul(
                        av[:, 0:d + 1],
                        expT_[kb][:, qb * 128:(qb + 1) * 128],
                        vp[:, kb, :],
                        start=(kb == 0),
                        stop=(kb == qb),
                    )
                rec = small.tile([128, 1], F32, tag="rec")
                nc.vector.reciprocal(rec, av[:, d:d + 1])
                nc.vector.tensor_scalar_mul(ostage[:, qb, :], av[:, 0:d], rec)

            last = bh == nbh - 1
            # AVs for strips this head inherited from the previous head's
            # tail: their exps are in flight or done, and these cheap early
            # AV blocks fill the PE while ScalarE chews the long strips.
            for qb in range(max(pre_done - 1, 0)):
                av_block(qb)
            for t in range(pre_done, SB):
                do_strip(t, qkt, expT)
                if t >= 1:
                    av_block(t - 1)  # one step behind so PE never waits on exp
                # NOTE: heads after the first start this loop at
                # t=pre_done(=4), so all slots here must be >= 4.
                # Stores before the transposes: the xbar barrier waits
                # out every in-flight DMA of the sync queue.
                if t == 4 and pending_vcast[0] is not None:
                    vpp, vnn = pending_vcast[0]
                    pending_vcast[0] = None
                    nc.vector.tensor_copy(vpp[:, :, 0:d], vnn)
                if prev is not None:
                    if t == 4:
                        store_chunk(prev[0], prev[1], 0, SB // 2)
                    elif t == 6:
                        store_chunk(prev[0], prev[1], SB // 2, SB)
                if nxt_ld is not None:
                    if t == 8:
                        nxt = transposes(nxt_ld)
                # last head: chunked stores right behind the AV drain so
                # the final store is tiny.  At step t, av_block(t-1) has
                # been emitted, so blocks [0, t) of ostage are in flight.
                if last:
                    if t == 9:
                        store_chunk(bh, ostage, 0, SB // 2)
                    elif t == 15:
                        store_chunk(bh, ostage, SB // 2, SB - 1)
            # tail: the next head's first NPRE strips go out BEFORE the
            # last AV so their (long) exps overlap this head's AV drain;
            # they write the OTHER epool2 buffers, so no clash with this
            # head's remaining AV reads of expT[0..NPRE-1].
            if not last:
                expT_next = make_expT(bh + 1)
                for t in range(NPRE):
                    do_strip(t, nxt[0], expT_next)
                expT, pre_done = expT_next, NPRE
            av_block(SB - 1)
            if last:
                store_chunk(bh, ostage, SB - 1, SB)
            prev = (bh, ostage)

    _legalize_waits(nc)
    return nc


_nc_cache = {}


def _get_nc():
    key = (NBH, S, D)
    if key not in _nc_cache:
        _nc_cache[key] = build_nc()
    return _nc_cache[key]


def kernel(query, key, value, ctx_mask):
    q = np.ascontiguousarray(query, dtype=np.float32).reshape(B * H, S, D)
    k = np.ascontiguousarray(key, dtype=np.float32).reshape(B * H, S, D)
    v = np.ascontiguousarray(value, dtype=np.float32).reshape(B * H, S, D)
    cmf = np.ascontiguousarray(ctx_mask, dtype=np.float32)

    in_maps = []
    for c in range(NCORES):
        lo = c * NBH
        in_maps.append({
            "q": q[lo:lo + NBH],
            "k": k[lo:lo + NBH],
            "v": v[lo:lo + NBH],
            "cm": cmf[(lo // H)],
        })
    nc = _get_nc()
    res = run_bass_kernel_spmd(nc, in_maps, list(range(NCORES)))
    outs = [res.results[c]["out"] for c in range(NCORES)]
    return np.concatenate(outs, axis=0).reshape(B, H, S, D).astype(np.float32)
